# revision 2
# baseline (speedup 1.0000x reference)
"""Cost-volume kernel for Trainium2 (8 NeuronCores, Bass).

cost[b, i, h, w] = mean_c f1[b,c,h,w] * f2[b,c,h,w-i]  (0 where w < i)

Strategy per (b, h) plane (C=128 on partitions):
  f2r[c, v] = f2[c, 255-v] (DVE reverse copy), zeros for v in [256, 320)
  H2[w, v]  = sum_c f1[c, w] * f2r[c, v]   (PE, 2 matmul tiles -> one PSUM bank)
  Hcat      = H2 * (1/128)                 (ACT copy PSUM->SBUF)
  band:  out[j, w] = H2[w, 255-w+j] -> anti-diagonal DMA (step row-1) into PK
  PE transpose (PK^T @ I) -> Tt[j, w] = out plane; DVE copy PSUM->SBUF; DMA out.

Sharding: 8 cores x 16 H-rows (data-parallel over B*H planes, 64 planes/core).
"""
import numpy as np

import concourse.bass as bass
import concourse.mybir as mybir
from concourse.bass_utils import run_bass_kernel_spmd

B, C, H, W = 4, 128, 128, 256
L = 64
NCORES = 8
HS = H // NCORES          # 16 h-rows per core
NPL = B * HS              # 64 planes per core
NB = 3                    # SBUF buffer depth (F1/F2/F2R/HC)
NPH = 2                   # PSUM banks for H2
NPT = 2                   # PSUM banks for transpose output
NPK = 2                   # PK buffers
NT2 = 3                   # T2 buffers

F32 = mybir.dt.float32


def _build(nc_holder={}):
    if "nc" in nc_holder:
        return nc_holder["nc"]
    nc = bass.Bass()
    f1 = nc.dram_tensor("f1", [B, C, HS, W], F32, kind="ExternalInput")
    f2 = nc.dram_tensor("f2", [B, C, HS, W], F32, kind="ExternalInput")
    ident = nc.dram_tensor("ident", [128, 128], F32, kind="ExternalInput")
    out = nc.dram_tensor("out", [B, L, HS, W], F32, kind="ExternalOutput")

    from contextlib import ExitStack
    ctx = ExitStack()
    sL = ctx.enter_context(nc.semaphore("sL"))
    sS = ctx.enter_context(nc.semaphore("sS"))
    sO = ctx.enter_context(nc.semaphore("sO"))
    cR = ctx.enter_context(nc.semaphore("cR"))
    cZ = ctx.enter_context(nc.semaphore("cZ"))
    cM = ctx.enter_context(nc.semaphore("cM"))
    cH = ctx.enter_context(nc.semaphore("cH"))
    cT = ctx.enter_context(nc.semaphore("cT"))
    cV = ctx.enter_context(nc.semaphore("cV"))
    I = ctx.enter_context(nc.sbuf_tensor("I", [128, 128], F32))
    F1 = [ctx.enter_context(nc.sbuf_tensor(f"F1_{k}", [128, 256], F32)) for k in range(NB)]
    F2 = [ctx.enter_context(nc.sbuf_tensor(f"F2_{k}", [128, 256], F32)) for k in range(NB)]
    F2R = [ctx.enter_context(nc.sbuf_tensor(f"F2R_{k}", [128, 320], F32)) for k in range(NB)]
    HC = [ctx.enter_context(nc.sbuf_tensor(f"HC_{k}", [128, 384], F32)) for k in range(NB)]
    PK = [ctx.enter_context(nc.sbuf_tensor(f"PK_{k}", [128, 128], F32)) for k in range(NPK)]
    T2 = [ctx.enter_context(nc.sbuf_tensor(f"T2_{k}", [64, 256], F32)) for k in range(NT2)]
    Hp = [ctx.enter_context(nc.psum_tensor(f"Hp_{k}", [128, 384], F32)) for k in range(NPH)]
    Tt = [ctx.enter_context(nc.psum_tensor(f"Tt_{k}", [64, 256], F32)) for k in range(NPT)]

    def f1_plane(i):
        b, hl = i // HS, i % HS
        return bass.AP(f1, (b * C * HS + hl) * W, [[HS * W, 128], [1, W]])

    def f2_plane(i):
        b, hl = i // HS, i % HS
        return bass.AP(f2, (b * C * HS + hl) * W, [[HS * W, 128], [1, W]])

    def out_plane(i):
        b, hl = i // HS, i % HS
        return bass.AP(out, (b * L * HS + hl) * W, [[HS * W, 64], [1, W]])

    with nc.Block() as block:

        @block.sync
        def _(sync):
            sync.dma_start(I[:, :], ident[:, :]).then_inc(sL, 16)
            for i in range(NPL):
                if i >= NB:
                    sync.wait_ge(cM, 2 * (i - NB + 1))   # F1 slot free (mm of i-NB done)
                    sync.wait_ge(cR, i - NB + 1)         # F2 slot free (revcopy done)
                sync.dma_start(F1[i % NB][:, :], f1_plane(i)).then_inc(sL, 16)
                sync.dma_start(F2[i % NB][:, :], f2_plane(i)).then_inc(sL, 16)
                if i >= 1:
                    sync.wait_ge(cV, i)                  # T2 of plane i-1 ready
                    sync.dma_start(out_plane(i - 1), T2[(i - 1) % NT2][:, :]).then_inc(sO, 16)
            sync.wait_ge(cV, NPL)
            sync.dma_start(out_plane(NPL - 1), T2[(NPL - 1) % NT2][:, :]).then_inc(sO, 16)

        @block.gpsimd
        def _(gpsimd):
            for i in range(NPL):
                if i >= NB:
                    gpsimd.wait_ge(cM, 2 * (i - NB + 1))  # F2R slot free
                gpsimd.memset(F2R[i % NB][:, 256:320], 0.0).then_inc(cZ, 1)

        @block.vector
        def _(vector):
            for i in range(NPL):
                if i >= NB:
                    vector.wait_ge(cM, 2 * (i - NB + 1))  # F2R slot free
                vector.wait_ge(sL, 16 + 32 * (i + 1))     # F2(i) loaded
                vector.tensor_copy(
                    F2R[i % NB][:, 0:256],
                    bass.AP(F2[i % NB], 255, [[256, 128], [-1, 256]]),
                ).then_inc(cR, 1)
                if i >= 1:
                    j = i - 1
                    if j >= NT2:
                        vector.wait_ge(sO, 16 * (j - NT2 + 1))  # T2 slot free
                    vector.wait_ge(cT, 2 * (j + 1))             # transposes(j) done
                    vector.tensor_copy(T2[j % NT2][:, :], Tt[j % NPT][:, :]).then_inc(cV, 1)
            j = NPL - 1
            vector.wait_ge(sO, 16 * (j - NT2 + 1))
            vector.wait_ge(cT, 2 * (j + 1))
            vector.tensor_copy(T2[j % NT2][:, :], Tt[j % NPT][:, :]).then_inc(cV, 1)

        @block.tensor
        def _(tensor):
            for i in range(NPL):
                tensor.wait_ge(sL, 16 + 32 * i + 16)   # F1(i) loaded
                tensor.wait_ge(cZ, i + 1)              # F2R zeros
                tensor.wait_ge(cR, i + 1)              # F2R reverse
                if i >= NPH:
                    tensor.wait_ge(cH, i - NPH + 1)    # Hp slot free
                tensor.matmul(Hp[i % NPH][:, 0:192], F1[i % NB][:, 0:128],
                              F2R[i % NB][:, 128:320]).then_inc(cM, 1)
                tensor.matmul(Hp[i % NPH][:, 192:384], F1[i % NB][:, 128:256],
                              F2R[i % NB][:, 0:192]).then_inc(cM, 1)
                if i >= 1:
                    j = i - 1
                    if j >= NPT:
                        tensor.wait_ge(cV, j - NPT + 1)  # Tt slot free
                    tensor.wait_ge(sS, 32 * (j + 1))     # shears(j) done
                    tensor.matmul(Tt[j % NPT][:, 0:128], PK[j % NPK][:, 0:64], I[:, :]).then_inc(cT, 1)
                    tensor.matmul(Tt[j % NPT][:, 128:256], PK[j % NPK][:, 64:128], I[:, :]).then_inc(cT, 1)
            j = NPL - 1
            tensor.wait_ge(cV, j - NPT + 1)
            tensor.wait_ge(sS, 32 * (j + 1))
            tensor.matmul(Tt[j % NPT][:, 0:128], PK[j % NPK][:, 0:64], I[:, :]).then_inc(cT, 1)
            tensor.matmul(Tt[j % NPT][:, 128:256], PK[j % NPK][:, 64:128], I[:, :]).then_inc(cT, 1)

        @block.scalar
        def _(scalar):
            for i in range(NPL):
                if i >= NB:
                    scalar.wait_ge(sS, 32 * (i - NB + 1))  # HC slot free (shears i-NB done)
                scalar.wait_ge(cM, 2 * (i + 1))            # mms(i) done
                scalar.mul(HC[i % NB][:, :], Hp[i % NPH][:, :], 1.0 / 128.0).then_inc(cH, 1)
                scalar.wait_ge(cH, i + 1)                  # own copy drained
                if i >= NPK:
                    scalar.wait_ge(cT, 2 * (i - NPK + 1))  # PK slot free
                scalar.dma_start(
                    bass.AP(PK[i % NPK], 0, [[128, 128], [1, 64]]),
                    bass.AP(HC[i % NB], 127, [[383, 128], [1, 64]]),
                ).then_inc(sS, 16)
                scalar.dma_start(
                    bass.AP(PK[i % NPK], 64, [[128, 128], [1, 64]]),
                    bass.AP(HC[i % NB], 319, [[383, 128], [1, 64]]),
                ).then_inc(sS, 16)

    nc_holder["nc"] = nc
    return nc


def run_sharded(features_1: np.ndarray, features_2: np.ndarray, **spmd_kwargs):
    """Shard over H, run on 8 cores, return (full_output, BassKernelResults)."""
    nc = _build()
    ident = np.eye(128, dtype=np.float32)
    in_maps = []
    for k in range(NCORES):
        sl = slice(k * HS, (k + 1) * HS)
        in_maps.append({
            "f1": np.ascontiguousarray(features_1[:, :, sl, :], dtype=np.float32),
            "f2": np.ascontiguousarray(features_2[:, :, sl, :], dtype=np.float32),
            "ident": ident,
        })
    res = run_bass_kernel_spmd(nc, in_maps, core_ids=list(range(NCORES)), **spmd_kwargs)
    full = np.empty((B, L, H, W), dtype=np.float32)
    for k in range(NCORES):
        full[:, :, k * HS:(k + 1) * HS, :] = res.results[k]["out"]
    return full, res


def kernel(features_1, features_2, lvls) -> np.ndarray:
    assert int(lvls) == L
    f1 = np.asarray(features_1, dtype=np.float32)
    f2 = np.asarray(features_2, dtype=np.float32)
    full, _ = run_sharded(f1, f2)
    return full


# revision 4
# speedup vs baseline: 2.8444x; 2.8444x over previous
"""Cost-volume kernel for Trainium2 (8 NeuronCores, Bass).

cost[b, i, h, w] = mean_c f1[b,c,h,w] * f2[b,c,h,w-i]  (0 where w < i)

Per (b, h) plane (C=128 on partitions):
  f2r[c, v] = f2[c, 255-v] (DVE reverse copy), zeros for v in [256, 320)
  H2[w, v]  = sum_c f1[c, w] * f2r[c, v]       (PE, 2 matmul tiles -> one PSUM bank)
  Hcat      = H2 * (1/128)                     (ACT copy PSUM->SBUF)
  band: out[j, w] = H2[w, 255-w+j]  -> anti-diagonal DMA (step row-1 = 383) into PK
  PE transpose (PK^T @ I) -> Tt[j, w] = output plane; DVE copy PSUM->SBUF; DMA out.

Pipeline: stage-lagged software pipeline; engine's i-th iteration touches plane
i-lag(stage) so cross-engine waits are satisfied in steady state.
DMA rings: SP ring = f1 loads + shear1; ACT ring = f2 loads + shear2;
Pool/SWDGE = output stores.  DMA completion semaphores are PER BUFFER SLOT so
each semaphore has at most one DMA in flight (unambiguous wait values).

Sharding: 8 cores x 16 H-rows (data-parallel over B*H planes, 64 planes/core).
"""
import numpy as np

import concourse.bass as bass
import concourse.mybir as mybir
from concourse.bass_utils import run_bass_kernel_spmd

B, C, H, W = 4, 128, 128, 256
L = 64
NCORES = 8
HS = H // NCORES          # 16 h-rows per core
NPL = B * HS              # 64 planes per core

# stage lags (plane index handled at engine-iteration i)
LAG_REV = 1      # DVE reverse copy
LAG_MM = 2       # PE gram matmuls
LAG_HC = 3       # ACT psum->sbuf copy, + shear DMAs
LAG_TT = 4       # PE transposes
LAG_T2 = 5       # DVE psum->sbuf copy of transposed plane
LAG_OUT = 6      # gpsimd SWDGE output store
NIT = NPL + LAG_OUT + 1

NB = 6            # F1/F2/F2R buffers
NHC = 4           # HC buffers
NPK = 4           # PK buffers
NT2 = 5           # T2 buffers
NPH = 3           # PSUM banks for H2
NPT = 3           # PSUM banks for transpose out

F32 = mybir.dt.float32


def _build(nc_holder={}):
    if "nc" in nc_holder:
        return nc_holder["nc"]
    nc = bass.Bass()
    f1 = nc.dram_tensor("f1", [B, C, HS, W], F32, kind="ExternalInput")
    f2 = nc.dram_tensor("f2", [B, C, HS, W], F32, kind="ExternalInput")
    ident = nc.dram_tensor("ident", [128, 128], F32, kind="ExternalInput")
    out = nc.dram_tensor("out", [B, L, HS, W], F32, kind="ExternalOutput")

    from contextlib import ExitStack
    ctx = ExitStack()
    sem = lambda n: ctx.enter_context(nc.semaphore(n))
    sbuf = lambda n, s: ctx.enter_context(nc.sbuf_tensor(n, s, F32))
    psum = lambda n, s: ctx.enter_context(nc.psum_tensor(n, s, F32))

    sI = sem("sI")                                   # ident load
    sF1 = [sem(f"sF1_{k}") for k in range(NB)]       # f1 loads (SP ring)
    sF2 = [sem(f"sF2_{k}") for k in range(NB)]       # f2 loads (ACT ring)
    sS1 = [sem(f"sS1_{k}") for k in range(NPK)]      # shear1 (SP ring)
    sS2 = [sem(f"sS2_{k}") for k in range(NPK)]      # shear2 (ACT ring)
    sO = [sem(f"sO_{k}") for k in range(NT2)]        # out stores (SWDGE)
    cR = sem("cR")     # revcopy, +1/plane
    cZ = sem("cZ")     # memset, +1/plane
    cM = sem("cM")     # gram mms, +2/plane
    cH = sem("cH")     # HC copy, +1/plane
    cT = sem("cT")     # transposes, +2/plane
    cV = sem("cV")     # T2 copy, +1/plane

    I = sbuf("I", [128, 128])
    F1 = [sbuf(f"F1_{k}", [128, 256]) for k in range(NB)]
    F2 = [sbuf(f"F2_{k}", [128, 256]) for k in range(NB)]
    F2R = [sbuf(f"F2R_{k}", [128, 320]) for k in range(NB)]
    HC = [sbuf(f"HC_{k}", [128, 384]) for k in range(NHC)]
    PK = [sbuf(f"PK_{k}", [128, 128]) for k in range(NPK)]
    T2 = [sbuf(f"T2_{k}", [64, 256]) for k in range(NT2)]
    Hp = [psum(f"Hp_{k}", [128, 384]) for k in range(NPH)]
    Tt = [psum(f"Tt_{k}", [64, 256]) for k in range(NPT)]

    # wait value for the j-th use of a per-slot DMA sem (16 per completed DMA)
    uses = lambda j, n: 16 * (j // n + 1)

    def f1_plane(j):
        b, hl = j // HS, j % HS
        return bass.AP(f1, (b * C * HS + hl) * W, [[HS * W, 128], [1, W]])

    def f2_plane(j):
        b, hl = j // HS, j % HS
        return bass.AP(f2, (b * C * HS + hl) * W, [[HS * W, 128], [1, W]])

    def out_plane(j):
        b, hl = j // HS, j % HS
        return bass.AP(out, (b * L * HS + hl) * W, [[HS * W, 64], [1, W]])

    def shear_src(j, half):
        # packed[w, j'] = HC[w, (127 or 319) - w + j'] ; anti-diag step 383
        return bass.AP(HC[j % NHC], 127 + 192 * half, [[383, 128], [1, 64]])

    with nc.Block() as block:

        @block.sync
        def _(sync):
            sync.dma_start(I[:, :], ident[:, :]).then_inc(sI, 16)
            for i in range(NIT):
                j = i
                if j < NPL:
                    # F1 slot free: gram mms of plane j-NB done
                    if j >= NB:
                        sync.wait_ge(cM, 2 * (j - NB + 1))
                    sync.dma_start(F1[j % NB][:, :], f1_plane(j)).then_inc(sF1[j % NB], 16)
                j = i - LAG_HC
                if 0 <= j < NPL:
                    sync.wait_ge(cH, j + 1)              # HC(j) written
                    if j >= NPK:
                        sync.wait_ge(cT, 2 * (j - NPK + 1))  # PK slot free
                    sync.dma_start(
                        bass.AP(PK[j % NPK], 0, [[128, 128], [1, 64]]),
                        shear_src(j, 0),
                    ).then_inc(sS1[j % NPK], 16)

        @block.scalar
        def _(scalar):
            for i in range(NIT):
                j = i
                if j < NPL:
                    if j >= NB:
                        scalar.wait_ge(cM, 2 * (j - NB + 1))  # F2 slot free (mm done)
                        scalar.wait_ge(cR, j - NB + 1)        # (revcopy done)
                    scalar.dma_start(F2[j % NB][:, :], f2_plane(j)).then_inc(sF2[j % NB], 16)
                j = i - LAG_HC
                if 0 <= j < NPL:
                    scalar.wait_ge(cM, 2 * (j + 1))      # gram mms(j) done
                    if j >= NHC:
                        jj = j - NHC                     # HC slot's previous user
                        scalar.wait_ge(sS1[jj % NPK], uses(jj, NPK))
                        scalar.wait_ge(sS2[jj % NPK], uses(jj, NPK))
                    scalar.mul(HC[j % NHC][:, :], Hp[j % NPH][:, :], 1.0 / 128.0).then_inc(cH, 1)
                    scalar.wait_ge(cH, j + 1)            # own copy drained
                    if j >= NPK:
                        scalar.wait_ge(cT, 2 * (j - NPK + 1))  # PK slot free
                    scalar.dma_start(
                        bass.AP(PK[j % NPK], 64, [[128, 128], [1, 64]]),
                        shear_src(j, 1),
                    ).then_inc(sS2[j % NPK], 16)

        @block.gpsimd
        def _(gpsimd):
            for i in range(NIT):
                j = i
                if j < NPL:
                    if j >= NB:
                        gpsimd.wait_ge(cM, 2 * (j - NB + 1))  # F2R slot free
                    gpsimd.memset(F2R[j % NB][:, 256:320], 0.0).then_inc(cZ, 1)
                j = i - LAG_OUT
                if 0 <= j < NPL:
                    gpsimd.wait_ge(cV, j + 1)            # T2(j) ready
                    gpsimd.dma_start(out_plane(j), T2[j % NT2][:, :]).then_inc(sO[j % NT2], 16)

        @block.vector
        def _(vector):
            for i in range(NIT):
                j = i - LAG_REV
                if 0 <= j < NPL:
                    if j >= NB:
                        vector.wait_ge(cM, 2 * (j - NB + 1))  # F2R slot free
                    vector.wait_ge(cZ, j + 1)                 # memset done (same slot)
                    vector.wait_ge(sF2[j % NB], uses(j, NB))  # F2(j) loaded
                    vector.tensor_copy(
                        F2R[j % NB][:, 0:256],
                        bass.AP(F2[j % NB], 255, [[256, 128], [-1, 256]]),
                    ).then_inc(cR, 1)
                j = i - LAG_T2
                if 0 <= j < NPL:
                    if j >= NT2:
                        jj = j - NT2
                        vector.wait_ge(sO[jj % NT2], uses(jj, NT2))  # T2 slot free
                    vector.wait_ge(cT, 2 * (j + 1))             # transposes(j) done
                    vector.tensor_copy(T2[j % NT2][:, :], Tt[j % NPT][:, :]).then_inc(cV, 1)

        @block.tensor
        def _(tensor):
            for i in range(NIT):
                j = i - LAG_MM
                if 0 <= j < NPL:
                    tensor.wait_ge(sF1[j % NB], uses(j, NB))  # F1(j) loaded
                    tensor.wait_ge(cR, j + 1)                 # f2r ready
                    if j >= NPH:
                        tensor.wait_ge(cH, j - NPH + 1)       # Hp slot free
                    tensor.matmul(Hp[j % NPH][:, 0:192], F1[j % NB][:, 0:128],
                                  F2R[j % NB][:, 128:320]).then_inc(cM, 1)
                    tensor.matmul(Hp[j % NPH][:, 192:384], F1[j % NB][:, 128:256],
                                  F2R[j % NB][:, 0:192]).then_inc(cM, 1)
                j = i - LAG_TT
                if 0 <= j < NPL:
                    if j == 0:
                        tensor.wait_ge(sI, 16)                # identity loaded
                    if j >= NPT:
                        tensor.wait_ge(cV, j - NPT + 1)       # Tt slot free
                    tensor.wait_ge(sS1[j % NPK], uses(j, NPK))  # shear1(j) done
                    tensor.wait_ge(sS2[j % NPK], uses(j, NPK))  # shear2(j) done
                    tensor.matmul(Tt[j % NPT][:, 0:128], PK[j % NPK][:, 0:64],
                                  I[:, :]).then_inc(cT, 1)
                    tensor.matmul(Tt[j % NPT][:, 128:256], PK[j % NPK][:, 64:128],
                                  I[:, :]).then_inc(cT, 1)

    nc_holder["nc"] = nc
    return nc


def run_sharded(features_1: np.ndarray, features_2: np.ndarray, **spmd_kwargs):
    """Shard over H, run on 8 cores, return (full_output, BassKernelResults)."""
    nc = _build()
    ident = np.eye(128, dtype=np.float32)
    in_maps = []
    for k in range(NCORES):
        sl = slice(k * HS, (k + 1) * HS)
        in_maps.append({
            "f1": np.ascontiguousarray(features_1[:, :, sl, :], dtype=np.float32),
            "f2": np.ascontiguousarray(features_2[:, :, sl, :], dtype=np.float32),
            "ident": ident,
        })
    res = run_bass_kernel_spmd(nc, in_maps, core_ids=list(range(NCORES)), **spmd_kwargs)
    full = np.empty((B, L, H, W), dtype=np.float32)
    for k in range(NCORES):
        full[:, :, k * HS:(k + 1) * HS, :] = res.results[k]["out"]
    return full, res


def kernel(features_1, features_2, lvls) -> np.ndarray:
    assert int(lvls) == L
    f1 = np.asarray(features_1, dtype=np.float32)
    f2 = np.asarray(features_2, dtype=np.float32)
    full, _ = run_sharded(f1, f2)
    return full


# revision 5
# speedup vs baseline: 4.1675x; 1.4652x over previous
"""Cost-volume kernel for Trainium2 (8 NeuronCores, Bass).

cost[b, i, h, w] = mean_c f1[b,c,h,w] * f2[b,c,h,w-i]  (0 where w < i)

Per (b, h) plane (C=128 on partitions):
  f2r[c, v] = f2[c, 255-v] (DVE reverse copy), zeros for v in [256, 320)
  H2[w, v]  = sum_c f1[c, w] * f2r[c, v]      (PE, 2 matmul tiles)
  band: out[j, w] = H2[w, 255-w+j]            (anti-diagonal DMA, step row-1)
  PE transpose (PK^T @ (I/128)) -> Tt[j, w] = output plane (scale folded in)
  DVE copy PSUM->SBUF; DMA out.

Planes are processed in PAIRS (same batch b, adjacent h) to halve DMA count:
loads/revcopy/memset/HCcopy/shears/T2copy/out all operate on two planes per op.
Stage-lagged software pipeline; per-buffer-slot DMA semaphores (single DMA in
flight per semaphore).  DMA rings: SP = f1 loads + shear1; ACT = f2 loads +
shear2; Pool/SWDGE = output stores.

Sharding: 8 cores x 16 H-rows (data-parallel over B*H planes, 64 planes/core).
"""
import numpy as np

import concourse.bass as bass
import concourse.mybir as mybir
from concourse.bass_utils import run_bass_kernel_spmd

B, C, H, W = 4, 128, 128, 256
L = 64
NCORES = 8
HS = H // NCORES          # 16 h-rows per core
NPL = B * HS              # 64 planes per core
NPR = NPL // 2            # 32 pairs per core

# stage lags (pair index handled at engine-iteration i)
LAG_REV = 1
LAG_MM = 2
LAG_HC = 3
LAG_TT = 4
LAG_T2 = 5
LAG_OUT = 6
NIT = NPR + LAG_OUT + 1

NB = 4            # F1/F2/F2R pair buffers
NHC = 3           # HC pair buffers
NPK = 3           # PK pair buffers
NT2 = 3           # T2 pair buffers
NPH = 2           # PSUM pair slots for H2 (2 banks each)
NPT = 2           # PSUM pair slots for transpose out (1 bank each)

F32 = mybir.dt.float32


def _build(nc_holder={}):
    if "nc" in nc_holder:
        return nc_holder["nc"]
    nc = bass.Bass()
    f1 = nc.dram_tensor("f1", [B, C, HS, W], F32, kind="ExternalInput")
    f2 = nc.dram_tensor("f2", [B, C, HS, W], F32, kind="ExternalInput")
    ident = nc.dram_tensor("ident", [128, 128], F32, kind="ExternalInput")
    out = nc.dram_tensor("out", [B, L, HS, W], F32, kind="ExternalOutput")

    from contextlib import ExitStack
    ctx = ExitStack()
    sem = lambda n: ctx.enter_context(nc.semaphore(n))
    sbuf = lambda n, s: ctx.enter_context(nc.sbuf_tensor(n, s, F32))
    psum = lambda n, s: ctx.enter_context(nc.psum_tensor(n, s, F32))

    sI = sem("sI")
    sF1 = [sem(f"sF1_{k}") for k in range(NB)]
    sF2 = [sem(f"sF2_{k}") for k in range(NB)]
    sS1 = [sem(f"sS1_{k}") for k in range(NPK)]
    sS2 = [sem(f"sS2_{k}") for k in range(NPK)]
    sO = [sem(f"sO_{k}") for k in range(NT2)]
    cR = sem("cR")     # revcopy, +1/pair
    cZ = sem("cZ")     # memset, +1/pair
    cM = sem("cM")     # gram mms, +4/pair
    cH = sem("cH")     # HC copy, +1/pair
    cT = sem("cT")     # transposes, +4/pair
    cV = sem("cV")     # T2 copy, +1/pair

    I = sbuf("I", [128, 128])
    F1 = [sbuf(f"F1_{k}", [128, 512]) for k in range(NB)]
    F2 = [sbuf(f"F2_{k}", [128, 512]) for k in range(NB)]
    F2R = [sbuf(f"F2R_{k}", [128, 640]) for k in range(NB)]
    HC = [sbuf(f"HC_{k}", [128, 768]) for k in range(NHC)]
    PK = [sbuf(f"PK_{k}", [128, 256]) for k in range(NPK)]
    T2 = [sbuf(f"T2_{k}", [64, 512]) for k in range(NT2)]
    Hp = [psum(f"Hp_{k}", [128, 1024]) for k in range(NPH)]
    Tt = [psum(f"Tt_{k}", [64, 512]) for k in range(NPT)]

    uses = lambda q, n: 16 * (q // n + 1)

    def f1_pair(q):
        b, hl = (2 * q) // HS, (2 * q) % HS
        return bass.AP(f1, (b * C * HS + hl) * W, [[HS * W, 128], [W, 2], [1, W]])

    def f2_pair(q):
        b, hl = (2 * q) // HS, (2 * q) % HS
        return bass.AP(f2, (b * C * HS + hl) * W, [[HS * W, 128], [W, 2], [1, W]])

    def out_pair(q):
        b, hl = (2 * q) // HS, (2 * q) % HS
        return bass.AP(out, (b * L * HS + hl) * W, [[HS * W, 64], [W, 2], [1, W]])

    def shear_src(q, half):
        # pair slot holds two 384-col plane sections; anti-diag step 767
        return bass.AP(HC[q % NHC], 127 + 192 * half, [[767, 128], [384, 2], [1, 64]])

    def shear_dst(q, half):
        return bass.AP(PK[q % NPK], 64 * half, [[256, 128], [128, 2], [1, 64]])

    with nc.Block() as block:

        @block.sync
        def _(sync):
            sync.dma_start(I[:, :], ident[:, :]).then_inc(sI, 16)
            for i in range(NIT):
                q = i
                if q < NPR:
                    if q >= NB:
                        sync.wait_ge(cM, 4 * (q - NB + 1))   # F1 slot free
                    sync.dma_start(F1[q % NB][:, :], f1_pair(q)).then_inc(sF1[q % NB], 16)
                q = i - LAG_HC
                if 0 <= q < NPR:
                    sync.wait_ge(cH, q + 1)                  # HC(q) written
                    if q >= NPK:
                        sync.wait_ge(cT, 4 * (q - NPK + 1))  # PK slot free
                    sync.dma_start(shear_dst(q, 0), shear_src(q, 0)).then_inc(sS1[q % NPK], 16)

        @block.scalar
        def _(scalar):
            for i in range(NIT):
                q = i
                if q < NPR:
                    if q >= NB:
                        scalar.wait_ge(cR, q - NB + 1)       # F2 slot free
                    scalar.dma_start(F2[q % NB][:, :], f2_pair(q)).then_inc(sF2[q % NB], 16)
                q = i - LAG_HC
                if 0 <= q < NPR:
                    scalar.wait_ge(cM, 4 * (q + 1))          # gram mms(q) done
                    if q >= NHC:
                        qq = q - NHC                         # HC slot's previous user
                        scalar.wait_ge(sS1[qq % NPK], uses(qq, NPK))
                        scalar.wait_ge(sS2[qq % NPK], uses(qq, NPK))
                    # Hp pair [128,1024]: planes at cols [0:384) and [512:896)
                    scalar.copy(
                        bass.AP(HC[q % NHC], 0, [[768, 128], [384, 2], [1, 384]]),
                        bass.AP(Hp[q % NPH], 0, [[1024, 128], [512, 2], [1, 384]]),
                    ).then_inc(cH, 1)
                    scalar.wait_ge(cH, q + 1)                # own copy drained
                    if q >= NPK:
                        scalar.wait_ge(cT, 4 * (q - NPK + 1))  # PK slot free
                    scalar.dma_start(shear_dst(q, 1), shear_src(q, 1)).then_inc(sS2[q % NPK], 16)

        @block.gpsimd
        def _(gpsimd):
            for i in range(NIT):
                q = i
                if q < NPR:
                    if q >= NB:
                        gpsimd.wait_ge(cM, 4 * (q - NB + 1))  # F2R slot free
                    gpsimd.memset(
                        bass.AP(F2R[q % NB], 256, [[640, 128], [320, 2], [1, 64]]), 0.0
                    ).then_inc(cZ, 1)
                q = i - LAG_OUT
                if 0 <= q < NPR:
                    gpsimd.wait_ge(cV, q + 1)                # T2(q) ready
                    gpsimd.dma_start(
                        out_pair(q),
                        bass.AP(T2[q % NT2], 0, [[512, 64], [256, 2], [1, 256]]),
                    ).then_inc(sO[q % NT2], 16)

        @block.vector
        def _(vector):
            for i in range(NIT):
                q = i - LAG_REV
                if 0 <= q < NPR:
                    if q >= NB:
                        vector.wait_ge(cM, 4 * (q - NB + 1))  # F2R slot free
                    vector.wait_ge(cZ, q + 1)
                    vector.wait_ge(sF2[q % NB], uses(q, NB))  # F2(q) loaded
                    vector.tensor_copy(
                        bass.AP(F2R[q % NB], 0, [[640, 128], [320, 2], [1, 256]]),
                        bass.AP(F2[q % NB], 255, [[512, 128], [256, 2], [-1, 256]]),
                    ).then_inc(cR, 1)
                q = i - LAG_T2
                if 0 <= q < NPR:
                    if q >= NT2:
                        qq = q - NT2
                        vector.wait_ge(sO[qq % NT2], uses(qq, NT2))  # T2 slot free
                    vector.wait_ge(cT, 4 * (q + 1))           # transposes(q) done
                    vector.tensor_copy(T2[q % NT2][:, :], Tt[q % NPT][:, :]).then_inc(cV, 1)

        @block.tensor
        def _(tensor):
            for i in range(NIT):
                q = i - LAG_MM
                if 0 <= q < NPR:
                    tensor.wait_ge(sF1[q % NB], uses(q, NB))  # F1(q) loaded
                    tensor.wait_ge(cR, q + 1)                 # F2R(q) ready
                    if q >= NPH:
                        tensor.wait_ge(cH, q - NPH + 1)       # Hp slot free
                    hp, f1t, f2r = Hp[q % NPH], F1[q % NB], F2R[q % NB]
                    # plane 0
                    tensor.matmul(hp[:, 0:192], f1t[:, 0:128], f2r[:, 128:320]).then_inc(cM, 1)
                    tensor.matmul(hp[:, 192:384], f1t[:, 128:256], f2r[:, 0:192]).then_inc(cM, 1)
                    # plane 1
                    tensor.matmul(hp[:, 512:704], f1t[:, 256:384], f2r[:, 448:640]).then_inc(cM, 1)
                    tensor.matmul(hp[:, 704:896], f1t[:, 384:512], f2r[:, 320:512]).then_inc(cM, 1)
                q = i - LAG_TT
                if 0 <= q < NPR:
                    if q == 0:
                        tensor.wait_ge(sI, 16)
                    if q >= NPT:
                        tensor.wait_ge(cV, q - NPT + 1)       # Tt slot free
                    tensor.wait_ge(sS1[q % NPK], uses(q, NPK))
                    tensor.wait_ge(sS2[q % NPK], uses(q, NPK))
                    tt, pk = Tt[q % NPT], PK[q % NPK]
                    tensor.matmul(tt[:, 0:128], pk[:, 0:64], I[:, :]).then_inc(cT, 1)
                    tensor.matmul(tt[:, 128:256], pk[:, 64:128], I[:, :]).then_inc(cT, 1)
                    tensor.matmul(tt[:, 256:384], pk[:, 128:192], I[:, :]).then_inc(cT, 1)
                    tensor.matmul(tt[:, 384:512], pk[:, 192:256], I[:, :]).then_inc(cT, 1)

    nc_holder["nc"] = nc
    return nc


def run_sharded(features_1: np.ndarray, features_2: np.ndarray, **spmd_kwargs):
    """Shard over H, run on 8 cores, return (full_output, BassKernelResults)."""
    nc = _build()
    ident = np.eye(128, dtype=np.float32) / 128.0   # mean folded into transpose
    in_maps = []
    for k in range(NCORES):
        sl = slice(k * HS, (k + 1) * HS)
        in_maps.append({
            "f1": np.ascontiguousarray(features_1[:, :, sl, :], dtype=np.float32),
            "f2": np.ascontiguousarray(features_2[:, :, sl, :], dtype=np.float32),
            "ident": ident,
        })
    res = run_bass_kernel_spmd(nc, in_maps, core_ids=list(range(NCORES)), **spmd_kwargs)
    full = np.empty((B, L, H, W), dtype=np.float32)
    for k in range(NCORES):
        full[:, :, k * HS:(k + 1) * HS, :] = res.results[k]["out"]
    return full, res


def kernel(features_1, features_2, lvls) -> np.ndarray:
    assert int(lvls) == L
    f1 = np.asarray(features_1, dtype=np.float32)
    f2 = np.asarray(features_2, dtype=np.float32)
    full, _ = run_sharded(f1, f2)
    return full


# revision 7
# speedup vs baseline: 5.1640x; 1.2391x over previous
"""Cost-volume kernel for Trainium2 (8 NeuronCores, Bass).

cost[b, i, h, w] = mean_c f1[b,c,h,w] * f2[b,c,h,w-i]  (0 where w < i)

Per (b, h) plane (C=128 on partitions), fp16 datapath / fp32 accumulation:
  f2r[c, v] = fp16(f2[c, 255-v]) (DVE reverse+cast), zeros for v in [256, 320)
  H2[w, v]  = sum_c f1[c, w] * f2r[c, v]      (PE fp16, 2 matmul tiles, fp32 PSUM)
  hc        = fp16(H2)                        (ACT copy PSUM->SBUF)
  band: out[j, w] = H2[w, 255-w+j]            (anti-diagonal DMA, step row-1)
  PE transpose (PK^T @ (I/128)) -> Tt[j, w] = output plane (fp32, scale folded)
  copy PSUM->SBUF (DVE/ACT alternating); DMA out (fp32).

Planes processed in PAIRS (same b, adjacent h).  Stage-lagged software
pipeline; per-buffer-slot DMA semaphores.  DMA rings:
  Pool/SWDGE (gpsimd): f1 loads with fp32->fp16 cast
  ACT ring:            f2 loads (fp32)
  SP ring:             shear1 + shear2 (fp16) + output stores (fp32)

Sharding: 8 cores x 16 H-rows (data-parallel over B*H planes, 64 planes/core).
"""
import numpy as np

import concourse.bass as bass
import concourse.mybir as mybir
from concourse.bass_utils import run_bass_kernel_spmd

B, C, H, W = 4, 128, 128, 256
L = 64
NCORES = 8
HS = H // NCORES          # 16 h-rows per core
NPL = B * HS              # 64 planes per core
NPR = NPL // 2            # 32 pairs per core

# stage lags (pair index handled at engine-iteration i)
LAG_REV = 1
LAG_MM = 2
LAG_HC = 3
LAG_SH = 4
LAG_TT = 5
LAG_T2 = 6
LAG_OUT = 7
NIT = NPR + LAG_OUT + 1

NB = 4            # F1/F2/F2R pair buffers
NHC = 3           # HC pair buffers
NPK = 3           # PK pair buffers
NT2 = 3           # T2 pair buffers
NPH = 2           # PSUM pair slots for H2 (1 bank each, fp32 [128,896->1024B..])
NPT = 2           # PSUM pair slots for transpose out (1 bank each)

F32 = mybir.dt.float32
F16 = mybir.dt.float16


def _build(nc_holder={}):
    if "nc" in nc_holder:
        return nc_holder["nc"]
    nc = bass.Bass()
    f1 = nc.dram_tensor("f1", [B, C, HS, W], F32, kind="ExternalInput")
    f2 = nc.dram_tensor("f2", [B, C, HS, W], F32, kind="ExternalInput")
    ident = nc.dram_tensor("ident", [128, 128], F16, kind="ExternalInput")
    out = nc.dram_tensor("out", [B, L, HS, W], F32, kind="ExternalOutput")

    from contextlib import ExitStack
    ctx = ExitStack()
    sem = lambda n: ctx.enter_context(nc.semaphore(n))
    sbuf = lambda n, s, dt: ctx.enter_context(nc.sbuf_tensor(n, s, dt))
    psum = lambda n, s: ctx.enter_context(nc.psum_tensor(n, s, F32))

    sI = sem("sI")
    sF1 = [sem(f"sF1_{k}") for k in range(NB)]
    sF2 = [sem(f"sF2_{k}") for k in range(NB)]
    sS1 = [sem(f"sS1_{k}") for k in range(NPK)]
    sS2 = [sem(f"sS2_{k}") for k in range(NPK)]
    sO = [sem(f"sO_{k}") for k in range(NT2)]
    cR = sem("cR")     # revcopy, +1/pair
    cZ = sem("cZ")     # memset, +1/pair
    cM = sem("cM")     # gram mms, +4/pair
    cH = sem("cH")     # HC copy, +1/pair
    cT = sem("cT")     # transposes, +4/pair
    cVe = sem("cVe")   # T2 copy even pairs (DVE), +1
    cVo = sem("cVo")   # T2 copy odd pairs (ACT), +1

    I = sbuf("I", [128, 128], F16)
    F1 = [sbuf(f"F1_{k}", [128, 512], F16) for k in range(NB)]
    F2 = [sbuf(f"F2_{k}", [128, 512], F32) for k in range(NB)]
    F2R = [sbuf(f"F2R_{k}", [128, 640], F16) for k in range(NB)]
    HC = [sbuf(f"HC_{k}", [128, 768], F16) for k in range(NHC)]
    PK = [sbuf(f"PK_{k}", [128, 256], F16) for k in range(NPK)]
    T2 = [sbuf(f"T2_{k}", [64, 512], F32) for k in range(NT2)]
    Hp = [psum(f"Hp_{k}", [128, 1024]) for k in range(NPH)]
    Tt = [psum(f"Tt_{k}", [64, 512]) for k in range(NPT)]

    uses = lambda q, n: 16 * (q // n + 1)

    def pair_base(q):
        b, hl = (2 * q) // HS, (2 * q) % HS
        return b, hl

    def f1_pair(q):
        b, hl = pair_base(q)
        return bass.AP(f1, (b * C * HS + hl) * W, [[HS * W, 128], [W, 2], [1, W]])

    def f2_pair(q):
        b, hl = pair_base(q)
        return bass.AP(f2, (b * C * HS + hl) * W, [[HS * W, 128], [W, 2], [1, W]])

    def out_pair(q):
        b, hl = pair_base(q)
        return bass.AP(out, (b * L * HS + hl) * W, [[HS * W, 64], [W, 2], [1, W]])

    def shear_src(q, half):
        return bass.AP(HC[q % NHC], 127 + 192 * half, [[767, 128], [384, 2], [1, 64]])

    def shear_dst(q, half):
        return bass.AP(PK[q % NPK], 64 * half, [[256, 128], [128, 2], [1, 64]])

    def t2_wait(engine, q):
        # wait until T2 copy of pair q done
        if q % 2 == 0:
            engine.wait_ge(cVe, q // 2 + 1)
        else:
            engine.wait_ge(cVo, q // 2 + 1)

    def t2_copy(engine, q):
        if q >= NT2:
            qq = q - NT2
            engine.wait_ge(sO[qq % NT2], uses(qq, NT2))    # T2 slot free
        engine.wait_ge(cT, 4 * (q + 1))                    # transposes(q) done
        copy_fn = getattr(engine, "tensor_copy", None) or engine.copy
        copy_fn(T2[q % NT2][:, :], Tt[q % NPT][:, :]).then_inc(
            cVe if q % 2 == 0 else cVo, 1)

    with nc.Block() as block:

        @block.sync
        def _(sync):
            sync.dma_start(I[:, :], ident[:, :]).then_inc(sI, 16)
            for i in range(NIT):
                q = i - LAG_SH
                if 0 <= q < NPR:
                    sync.wait_ge(cH, q + 1)                  # HC(q) written
                    if q >= NPK:
                        sync.wait_ge(cT, 4 * (q - NPK + 1))  # PK slot free
                    sync.dma_start(shear_dst(q, 0), shear_src(q, 0)).then_inc(sS1[q % NPK], 16)
                    sync.dma_start(shear_dst(q, 1), shear_src(q, 1)).then_inc(sS2[q % NPK], 16)
                q = i - LAG_OUT
                if 0 <= q < NPR:
                    t2_wait(sync, q)
                    sync.dma_start(
                        out_pair(q),
                        bass.AP(T2[q % NT2], 0, [[512, 64], [256, 2], [1, 256]]),
                    ).then_inc(sO[q % NT2], 16)

        @block.scalar
        def _(scalar):
            for i in range(NIT):
                q = i
                if q < NPR:
                    if q >= NB:
                        scalar.wait_ge(cR, q - NB + 1)       # F2 slot free
                    scalar.dma_start(F2[q % NB][:, :], f2_pair(q)).then_inc(sF2[q % NB], 16)
                q = i - LAG_HC
                if 0 <= q < NPR:
                    scalar.wait_ge(cM, 4 * (q + 1))          # gram mms(q) done
                    if q >= NHC:
                        qq = q - NHC                         # HC slot's previous user
                        scalar.wait_ge(sS1[qq % NPK], uses(qq, NPK))
                        scalar.wait_ge(sS2[qq % NPK], uses(qq, NPK))
                    scalar.copy(
                        bass.AP(HC[q % NHC], 0, [[768, 128], [384, 2], [1, 384]]),
                        bass.AP(Hp[q % NPH], 0, [[1024, 128], [512, 2], [1, 384]]),
                    ).then_inc(cH, 1)
                q = i - LAG_T2
                if 0 <= q < NPR and q % 2 == 1:
                    t2_copy(scalar, q)

        @block.gpsimd
        def _(gpsimd):
            for i in range(NIT):
                q = i
                if q < NPR:
                    if q >= NB:
                        gpsimd.wait_ge(cM, 4 * (q - NB + 1))  # F1/F2R slot free
                    gpsimd.dma_start(
                        bass.AP(F1[q % NB], 0, [[512, 128], [256, 2], [1, 256]]),
                        f1_pair(q),
                    ).then_inc(sF1[q % NB], 16)
                    gpsimd.memset(
                        bass.AP(F2R[q % NB], 256, [[640, 128], [320, 2], [1, 64]]), 0.0
                    ).then_inc(cZ, 1)

        @block.vector
        def _(vector):
            for i in range(NIT):
                q = i - LAG_REV
                if 0 <= q < NPR:
                    if q >= NB:
                        vector.wait_ge(cM, 4 * (q - NB + 1))  # F2R slot free
                    vector.wait_ge(cZ, q + 1)
                    vector.wait_ge(sF2[q % NB], uses(q, NB))  # F2(q) loaded
                    vector.tensor_copy(
                        bass.AP(F2R[q % NB], 0, [[640, 128], [320, 2], [1, 256]]),
                        bass.AP(F2[q % NB], 255, [[512, 128], [256, 2], [-1, 256]]),
                    ).then_inc(cR, 1)
                q = i - LAG_T2
                if 0 <= q < NPR and q % 2 == 0:
                    t2_copy(vector, q)

        @block.tensor
        def _(tensor):
            for i in range(NIT):
                q = i - LAG_MM
                if 0 <= q < NPR:
                    tensor.wait_ge(sF1[q % NB], uses(q, NB))  # F1(q) loaded
                    tensor.wait_ge(cR, q + 1)                 # F2R(q) ready
                    if q >= NPH:
                        tensor.wait_ge(cH, q - NPH + 1)       # Hp slot free
                    hp, f1t, f2r = Hp[q % NPH], F1[q % NB], F2R[q % NB]
                    tensor.matmul(hp[:, 0:192], f1t[:, 0:128], f2r[:, 128:320]).then_inc(cM, 1)
                    tensor.matmul(hp[:, 192:384], f1t[:, 128:256], f2r[:, 0:192]).then_inc(cM, 1)
                    tensor.matmul(hp[:, 512:704], f1t[:, 256:384], f2r[:, 448:640]).then_inc(cM, 1)
                    tensor.matmul(hp[:, 704:896], f1t[:, 384:512], f2r[:, 320:512]).then_inc(cM, 1)
                q = i - LAG_TT
                if 0 <= q < NPR:
                    if q == 0:
                        tensor.wait_ge(sI, 16)
                    if q >= NPT:
                        t2_wait(tensor, q - NPT)              # Tt slot free
                    tensor.wait_ge(sS1[q % NPK], uses(q, NPK))
                    tensor.wait_ge(sS2[q % NPK], uses(q, NPK))
                    tt, pk = Tt[q % NPT], PK[q % NPK]
                    tensor.matmul(tt[:, 0:128], pk[:, 0:64], I[:, :]).then_inc(cT, 1)
                    tensor.matmul(tt[:, 128:256], pk[:, 64:128], I[:, :]).then_inc(cT, 1)
                    tensor.matmul(tt[:, 256:384], pk[:, 128:192], I[:, :]).then_inc(cT, 1)
                    tensor.matmul(tt[:, 384:512], pk[:, 192:256], I[:, :]).then_inc(cT, 1)

    nc_holder["nc"] = nc
    return nc


def run_sharded(features_1: np.ndarray, features_2: np.ndarray, **spmd_kwargs):
    """Shard over H, run on 8 cores, return (full_output, BassKernelResults)."""
    nc = _build()
    ident = (np.eye(128, dtype=np.float32) / 128.0).astype(np.float16)
    in_maps = []
    for k in range(NCORES):
        sl = slice(k * HS, (k + 1) * HS)
        in_maps.append({
            "f1": np.ascontiguousarray(features_1[:, :, sl, :], dtype=np.float32),
            "f2": np.ascontiguousarray(features_2[:, :, sl, :], dtype=np.float32),
            "ident": ident,
        })
    res = run_bass_kernel_spmd(nc, in_maps, core_ids=list(range(NCORES)), **spmd_kwargs)
    full = np.empty((B, L, H, W), dtype=np.float32)
    for k in range(NCORES):
        full[:, :, k * HS:(k + 1) * HS, :] = res.results[k]["out"]
    return full, res


def kernel(features_1, features_2, lvls) -> np.ndarray:
    assert int(lvls) == L
    f1 = np.asarray(features_1, dtype=np.float32)
    f2 = np.asarray(features_2, dtype=np.float32)
    full, _ = run_sharded(f1, f2)
    return full
